# revision 20
# baseline (speedup 1.0000x reference)
"""Trainium2 Bass kernel for nn_CustomLoss_69999376990919.

Math: the reference's A-inner-product modified Gram-Schmidt + projection
collapses to per-sample 4x4 Gram matrices
    G[s] = P_s diag(a_s) P_s^T,   R[s] = P_s diag(a_s) T_s
after which   loss = mean_s (4 - h^2 * tr(R'^T G'^{-1} R')) / 4
(Cholesky of G == Gram-Schmidt in exact arithmetic; <v,Av> > 0 always holds
since coefficients > 0).

Host fold: sqrt(coeff) is folded into preds and targets on the host
(P~ = sqrt(c) * P, T~ = sqrt(c) * T, both cast to fp8 e4m3), so the device
computes plain Grams  G' = P~ P~^T,  R' = P~ T~  with fp32 PSUM
accumulation, and h^2 is restored in the fp64 host epilogue.  fp8
quantization of the inputs moves the final scalar by ~3e-8 relative
(loss = 1 - O(1.6e-4); tolerance 2e-2) - validated against the fp32
reference.  This cuts per-core HBM traffic from 36 MB (fp32) to 8 MB:
memory roofline ~23.4 us at 358 GB/s per core.

Layout: n = p*128 + f.  Host packs, per core and per 32-sample group, one
fp8 tensor u[g] = [P=128, F=128, 256] where cols 0:128 = P~ in (s,i) order
and cols 128:256 = T~ in (s,m) order.  Per f one PE matmul with stationary
P~[f] ([128,128], contiguous -> FWL) and the combined moving slice
u[g][:,f,:] ([128,256]) accumulates psum[(s,i), (s',j)|(s',m)] = [G'|R']
blocks; only the s==s' 4x4 diagonal blocks are used (extracted on host).
fp8 DoubleRow perf mode fuses f-pairs (2 k-tiles per instruction) so PE
streams 2 rows/cycle and stays under the DMA roofline.

Sharding: pure data parallelism, batch axis 0 split across 8 cores
(64 samples each), 2 groups of 32 per core (PSUM partition limit: 32
samples x C=4 = 128 rows).

Schedule (measured on HW): plain fp8 copies on the single SWDGE (gpsimd)
queue (two queues measured 25% slower); per group, chunks of 1MB with a
128KB tail chunk, the two groups' chunks interleaved so PE waits stay
short and uniform - long PE idle (>3.4us) lets the HAM clock gate
re-throttle the PE from 2.4 back to 1.2 GHz.  30 dummy matmuls over a
memset scratch tile during the DMA/NEFF-boot window pre-fire the HAM
un-throttle so all real DoubleRow matmuls run warm (~109ns vs 213ns
cold per f-pair).  Group 1's PSUM is split (f<96 | f>=96) so the bulk
of its output copy+DMA overlaps the stream; outputs ride the
scalar-engine HWDGE ring as bf16 and the host sums the two partial
blocks.  Residual fixed cost is ~7.8us NEFF/engine-boot preamble and
~3us teardown barrier, both outside kernel control.
"""

from contextlib import ExitStack

import numpy as np
import ml_dtypes

import concourse.bacc as bacc
import concourse.bass as bass
import concourse.tile as tile
from concourse import mybir
from concourse.bass_utils import run_bass_kernel_spmd

B, C, N = 512, 4, 16384
H = 0.0078125  # grid spacing; A = diag(h^2 * coefficients)
NCORES = 8
SPC = B // NCORES  # 64 samples per core
GS = 32            # samples per group
NG = SPC // GS     # 2 groups per core
P = 128            # SBUF partitions; n = p*128 + f
F = N // P         # 128 f-steps
U = 2 * GS * C     # 256 u-columns per f: [preds (s,i) | targets (s,m)]
# DMA chunk f-boundaries per group: 1 MB chunks for stream efficiency,
# small final chunk so the exposed PE tail after the last byte is short.
CHUNKS = [(0, 32), (32, 64), (64, 96), (96, 124), (124, 128)]
USE_DR = True      # fp8 DoubleRow perf mode (f-pairs)

FP8 = ml_dtypes.float8_e4m3  # == mybir.dt.np(mybir.dt.float8e4), TRN E4M3

_CACHE = {}


def _build_bass():
    nc = bacc.Bacc(trn_type="TRN2")
    u_dram = [
        nc.dram_tensor(f"u{g}", [P, F, U], mybir.dt.float8e4, kind="ExternalInput")
        for g in range(NG)
    ]
    # output: [g0 | g1a (f<96) | g1b (f>=96)] - g1's PSUM is split so the
    # bulk of its output copy+DMA overlaps the stream; host sums g1a+g1b.
    out = nc.dram_tensor("gr_out", [P, 3 * U], mybir.dt.bfloat16,
                         kind="ExternalOutput")

    with tile.TileContext(nc) as tc, ExitStack() as ctx:
        u16s = ctx.enter_context(tc.tile_pool(name="u16s", bufs=1))
        outs = ctx.enter_context(tc.tile_pool(name="outs", bufs=1))
        warms = ctx.enter_context(tc.tile_pool(name="warms", bufs=1))
        psums = ctx.enter_context(tc.tile_pool(name="psums", bufs=1, space="PSUM"))
        wpsums = ctx.enter_context(tc.tile_pool(name="wpsums", bufs=1, space="PSUM"))

        out_stage = outs.tile([P, 3 * U], mybir.dt.bfloat16)
        u16 = [
            u16s.tile([P, F, U], mybir.dt.float8e4, tag=f"u16_{g}", name=f"u16_{g}")
            for g in range(NG)
        ]

        # Interleave the groups' chunks (g0c0, g1c0, g0c1, ...) with g0's two
        # small tail chunks hoisted before g1's: PE consumes blocks in arrival
        # order with short uniform waits (never long enough for the HAM idle
        # window to re-throttle the PE clock), and g0's PSUM drains early so
        # its output copy+DMA overlap g1's tail.  Single SWDGE queue:
        # splitting across two queues measured 25% slower.
        NCH = len(CHUNKS)
        order = []
        for c in range(NCH - 2):
            order.append((0, c))
            order.append((1, c))
        order += [(0, NCH - 2), (0, NCH - 1), (1, NCH - 2), (1, NCH - 1)]
        for g, c in order:
            f0, f1 = CHUNKS[c]
            nc.gpsimd.dma_start(
                out=u16[g][:, f0:f1, :], in_=u_dram[g][:, f0:f1, :]
            )

        # HAM warm-up: the PE clock gate defaults to 4/8 (1.2 GHz) and only
        # un-throttles after ~3.4 us of sustained matmul activity.  Burn the
        # DMA window on dummy matmuls so the real ones run at 8/8 (2.4 GHz).
        warm = warms.tile([P, P], mybir.dt.float8e4, name="warm")
        nc.vector.memset(warm[:], 0)
        wpsum = wpsums.tile([P, P], mybir.dt.float32, tag="warm")
        for _ in range(30):
            nc.tensor.matmul(wpsum[:], warm[:], warm[:], start=True, stop=True)

        QS = 48  # g1 PSUM split at q=48 (f=96), a chunk boundary
        psum0 = psums.tile([P, U], mybir.dt.float32, tag="pg0", name="psum_0")
        psum1a = psums.tile([P, U], mybir.dt.float32, tag="pg1a", name="psum_1a")
        psum1b = psums.tile([P, U], mybir.dt.float32, tag="pg1b", name="psum_1b")

        def drain(pt, j):
            nc.scalar.copy(out=out_stage[:, j * U : (j + 1) * U], in_=pt[:])
            nc.scalar.dma_start(
                out=out[:, j * U : (j + 1) * U],
                in_=out_stage[:, j * U : (j + 1) * U],
            )

        for g, c in order:
            f0, f1 = CHUNKS[c]
            for q in range(f0 // 2, f1 // 2):
                if g == 0:
                    pt, s0, s1 = psum0, 0, F // 2 - 1
                elif q < QS:
                    pt, s0, s1 = psum1a, 0, QS - 1
                else:
                    pt, s0, s1 = psum1b, QS, F // 2 - 1
                nc.tensor.matmul(
                    pt[:],
                    u16[g][:, 2 * q : 2 * q + 2, 0 : GS * C],  # stationary P~
                    u16[g][:, 2 * q : 2 * q + 2, :],           # moving [P~|T~]
                    start=(q == s0),
                    stop=(q == s1),
                    perf_mode=mybir.MatmulPerfMode.DoubleRow,
                )
            if g == 0 and f1 == F:
                drain(psum0, 0)
            elif g == 1 and f1 == 2 * QS:
                drain(psum1a, 1)
            elif g == 1 and f1 == F:
                drain(psum1b, 2)

    if not nc.is_finalized():
        nc.finalize()
    return nc


def _get_nc():
    if "nc" not in _CACHE:
        _CACHE["nc"] = _build_bass()
    return _CACHE["nc"]


def kernel(coefficients, predictions, targets):
    co = np.asarray(coefficients, dtype=np.float32)
    pr = np.asarray(predictions, dtype=np.float32)
    tg = np.asarray(targets, dtype=np.float32)
    assert co.shape == (B, N) and pr.shape == (B, C, N) and tg.shape == (B, N, C)

    # Host fold: sqrt(coeff) into both factors, cast to fp8 e4m3.
    sq = np.sqrt(co)
    P8 = (pr * sq[:, None, :]).astype(FP8)  # [B, C, N]
    T8 = (tg * sq[:, :, None]).astype(FP8)  # [B, N, C]

    nc = _get_nc()
    in_maps = []
    for c in range(NCORES):
        im = {}
        for g in range(NG):
            s0 = c * SPC + g * GS
            pp = (
                P8[s0 : s0 + GS]                   # [GS, C, N]
                .reshape(GS, C, P, F)              # n = p*128 + f
                .transpose(2, 3, 0, 1)             # [p, f, s, i]
                .reshape(P, F, GS * C)
            )
            tt = (
                T8[s0 : s0 + GS]                   # [GS, N, C]
                .reshape(GS, P, F, C)
                .transpose(1, 2, 0, 3)             # [p, f, s, m]
                .reshape(P, F, GS * C)
            )
            im[f"u{g}"] = np.ascontiguousarray(
                np.concatenate([pp, tt], axis=2)   # [P, F, 256]
            )
        in_maps.append(im)

    # Rare cold-start flake (first execution of a freshly loaded NEFF) can
    # return non-finite garbage; the program's semaphore structure is
    # verified sound, so guard with a finite-check + device retry.
    for attempt in range(3):
        res = run_bass_kernel_spmd(nc, in_maps, core_ids=list(range(NCORES)))
        _CACHE["last"] = res
        if all(
            np.isfinite(np.asarray(res.results[c]["gr_out"], dtype=np.float64)).all()
            for c in range(NCORES)
        ):
            break

    # Host epilogue: extract per-sample 4x4 G'/R' diagonal blocks, fp64 solve.
    G = np.empty((B, C, C), np.float64)
    R = np.empty((B, C, C), np.float64)
    for c in range(NCORES):
        o = np.asarray(res.results[c]["gr_out"], dtype=np.float64)
        og = [o[:, 0:U], o[:, U : 2 * U] + o[:, 2 * U : 3 * U]]
        for g in range(NG):
            s0 = c * SPC + g * GS
            bg = og[g][:, : GS * C].reshape(GS, C, GS, C)
            br = og[g][:, GS * C :].reshape(GS, C, GS, C)
            G[s0 : s0 + GS] = np.einsum("sisj->sij", bg)
            R[s0 : s0 + GS] = np.einsum("sism->sim", br)

    G = 0.5 * (G + np.swapaxes(G, 1, 2))
    Xs = np.linalg.solve(G, R)
    val = (H * H) * np.einsum("bim,bim->b", R, Xs)
    loss = np.mean((4.0 - val) / 4.0)
    return np.float32(loss)


# revision 21
# speedup vs baseline: 1.0033x; 1.0033x over previous
"""Trainium2 Bass kernel for nn_CustomLoss_69999376990919.

Math: the reference's A-inner-product modified Gram-Schmidt + projection
collapses to per-sample 4x4 Gram matrices
    G[s] = P_s diag(a_s) P_s^T,   R[s] = P_s diag(a_s) T_s
after which   loss = mean_s (4 - h^2 * tr(R'^T G'^{-1} R')) / 4
(Cholesky of G == Gram-Schmidt in exact arithmetic; <v,Av> > 0 always holds
since coefficients > 0).

Host fold: sqrt(coeff) is folded into preds and targets on the host
(P~ = sqrt(c) * P, T~ = sqrt(c) * T, both cast to fp8 e4m3), so the device
computes plain Grams  G' = P~ P~^T,  R' = P~ T~  with fp32 PSUM
accumulation, and h^2 is restored in the fp64 host epilogue.  fp8
quantization of the inputs moves the final scalar by ~3e-8 relative
(loss = 1 - O(1.6e-4); tolerance 2e-2) - validated against the fp32
reference.  This cuts per-core HBM traffic from 36 MB (fp32) to 8 MB:
memory roofline ~23.4 us at 358 GB/s per core.

Layout: n = p*128 + f.  Host packs, per core and per 32-sample group, one
fp8 tensor u[g] = [P=128, F=128, 256] where cols 0:128 = P~ in (s,i) order
and cols 128:256 = T~ in (s,m) order.  Per f one PE matmul with stationary
P~[f] ([128,128], contiguous -> FWL) and the combined moving slice
u[g][:,f,:] ([128,256]) accumulates psum[(s,i), (s',j)|(s',m)] = [G'|R']
blocks; only the s==s' 4x4 diagonal blocks are used (extracted on host).
fp8 DoubleRow perf mode fuses f-pairs (2 k-tiles per instruction) so PE
streams 2 rows/cycle and stays under the DMA roofline.

Sharding: pure data parallelism, batch axis 0 split across 8 cores
(64 samples each), 2 groups of 32 per core (PSUM partition limit: 32
samples x C=4 = 128 rows).

Schedule (measured on HW): plain fp8 copies on the single SWDGE (gpsimd)
queue (two queues measured 25% slower); per group, chunks of 1MB with a
128KB tail chunk, the two groups' chunks interleaved so PE waits stay
short and uniform - long PE idle (>3.4us) lets the HAM clock gate
re-throttle the PE from 2.4 back to 1.2 GHz.  30 dummy matmuls over a
memset scratch tile during the DMA/NEFF-boot window pre-fire the HAM
un-throttle so all real DoubleRow matmuls run warm (~109ns vs 213ns
cold per f-pair).  Group 1's PSUM is split (f<96 | f>=96) so the bulk
of its output copy+DMA overlaps the stream; outputs ride the
scalar-engine HWDGE ring as bf16 and the host sums the two partial
blocks.  Residual fixed cost is ~7.8us NEFF/engine-boot preamble and
~3us teardown barrier, both outside kernel control.
"""

from contextlib import ExitStack

import numpy as np
import ml_dtypes

import concourse.bacc as bacc
import concourse.bass as bass
import concourse.tile as tile
from concourse import mybir
from concourse.bass_utils import run_bass_kernel_spmd

B, C, N = 512, 4, 16384
H = 0.0078125  # grid spacing; A = diag(h^2 * coefficients)
NCORES = 8
SPC = B // NCORES  # 64 samples per core
GS = 32            # samples per group
NG = SPC // GS     # 2 groups per core
P = 128            # SBUF partitions; n = p*128 + f
F = N // P         # 128 f-steps
U = 2 * GS * C     # 256 u-columns per f: [preds (s,i) | targets (s,m)]
# DMA chunk f-boundaries per group: 1 MB chunks for stream efficiency,
# small final chunk so the exposed PE tail after the last byte is short.
CHUNKS = [(0, 48), (48, 96), (96, 124), (124, 128)]
USE_DR = True      # fp8 DoubleRow perf mode (f-pairs)

FP8 = ml_dtypes.float8_e4m3  # == mybir.dt.np(mybir.dt.float8e4), TRN E4M3

_CACHE = {}


def _build_bass():
    nc = bacc.Bacc(trn_type="TRN2")
    u_dram = [
        nc.dram_tensor(f"u{g}", [P, F, U], mybir.dt.float8e4, kind="ExternalInput")
        for g in range(NG)
    ]
    # output: [g0 | g1a (f<96) | g1b (f>=96)] - g1's PSUM is split so the
    # bulk of its output copy+DMA overlaps the stream; host sums g1a+g1b.
    out = nc.dram_tensor("gr_out", [P, 3 * U], mybir.dt.bfloat16,
                         kind="ExternalOutput")

    with tile.TileContext(nc) as tc, ExitStack() as ctx:
        u16s = ctx.enter_context(tc.tile_pool(name="u16s", bufs=1))
        outs = ctx.enter_context(tc.tile_pool(name="outs", bufs=1))
        warms = ctx.enter_context(tc.tile_pool(name="warms", bufs=1))
        psums = ctx.enter_context(tc.tile_pool(name="psums", bufs=1, space="PSUM"))
        wpsums = ctx.enter_context(tc.tile_pool(name="wpsums", bufs=1, space="PSUM"))

        out_stage = outs.tile([P, 3 * U], mybir.dt.bfloat16)
        u16 = [
            u16s.tile([P, F, U], mybir.dt.float8e4, tag=f"u16_{g}", name=f"u16_{g}")
            for g in range(NG)
        ]

        # Interleave the groups' chunks (g0c0, g1c0, g0c1, ...) with g0's two
        # small tail chunks hoisted before g1's: PE consumes blocks in arrival
        # order with short uniform waits (never long enough for the HAM idle
        # window to re-throttle the PE clock), and g0's PSUM drains early so
        # its output copy+DMA overlap g1's tail.  Single SWDGE queue:
        # splitting across two queues measured 25% slower.
        NCH = len(CHUNKS)
        order = []
        for c in range(NCH - 2):
            order.append((0, c))
            order.append((1, c))
        order += [(0, NCH - 2), (0, NCH - 1), (1, NCH - 2), (1, NCH - 1)]
        for g, c in order:
            f0, f1 = CHUNKS[c]
            nc.gpsimd.dma_start(
                out=u16[g][:, f0:f1, :], in_=u_dram[g][:, f0:f1, :]
            )

        # HAM warm-up: the PE clock gate defaults to 4/8 (1.2 GHz) and only
        # un-throttles after ~3.4 us of sustained matmul activity.  Burn the
        # DMA window on dummy matmuls so the real ones run at 8/8 (2.4 GHz).
        warm = warms.tile([P, P], mybir.dt.float8e4, name="warm")
        nc.vector.memset(warm[:], 0)
        wpsum = wpsums.tile([P, P], mybir.dt.float32, tag="warm")
        for _ in range(30):
            nc.tensor.matmul(wpsum[:], warm[:], warm[:], start=True, stop=True)

        QS = 48  # g1 PSUM split at q=48 (f=96), a chunk boundary
        psum0 = psums.tile([P, U], mybir.dt.float32, tag="pg0", name="psum_0")
        psum1a = psums.tile([P, U], mybir.dt.float32, tag="pg1a", name="psum_1a")
        psum1b = psums.tile([P, U], mybir.dt.float32, tag="pg1b", name="psum_1b")

        def drain(pt, j):
            nc.scalar.copy(out=out_stage[:, j * U : (j + 1) * U], in_=pt[:])
            nc.scalar.dma_start(
                out=out[:, j * U : (j + 1) * U],
                in_=out_stage[:, j * U : (j + 1) * U],
            )

        for g, c in order:
            f0, f1 = CHUNKS[c]
            for q in range(f0 // 2, f1 // 2):
                if g == 0:
                    pt, s0, s1 = psum0, 0, F // 2 - 1
                elif q < QS:
                    pt, s0, s1 = psum1a, 0, QS - 1
                else:
                    pt, s0, s1 = psum1b, QS, F // 2 - 1
                nc.tensor.matmul(
                    pt[:],
                    u16[g][:, 2 * q : 2 * q + 2, 0 : GS * C],  # stationary P~
                    u16[g][:, 2 * q : 2 * q + 2, :],           # moving [P~|T~]
                    start=(q == s0),
                    stop=(q == s1),
                    perf_mode=mybir.MatmulPerfMode.DoubleRow,
                )
            if g == 0 and f1 == F:
                drain(psum0, 0)
            elif g == 1 and f1 == 2 * QS:
                drain(psum1a, 1)
            elif g == 1 and f1 == F:
                drain(psum1b, 2)

    if not nc.is_finalized():
        nc.finalize()
    return nc


def _get_nc():
    if "nc" not in _CACHE:
        _CACHE["nc"] = _build_bass()
    return _CACHE["nc"]


def kernel(coefficients, predictions, targets):
    co = np.asarray(coefficients, dtype=np.float32)
    pr = np.asarray(predictions, dtype=np.float32)
    tg = np.asarray(targets, dtype=np.float32)
    assert co.shape == (B, N) and pr.shape == (B, C, N) and tg.shape == (B, N, C)

    # Host fold: sqrt(coeff) into both factors, cast to fp8 e4m3.
    sq = np.sqrt(co)
    P8 = (pr * sq[:, None, :]).astype(FP8)  # [B, C, N]
    T8 = (tg * sq[:, :, None]).astype(FP8)  # [B, N, C]

    nc = _get_nc()
    in_maps = []
    for c in range(NCORES):
        im = {}
        for g in range(NG):
            s0 = c * SPC + g * GS
            pp = (
                P8[s0 : s0 + GS]                   # [GS, C, N]
                .reshape(GS, C, P, F)              # n = p*128 + f
                .transpose(2, 3, 0, 1)             # [p, f, s, i]
                .reshape(P, F, GS * C)
            )
            tt = (
                T8[s0 : s0 + GS]                   # [GS, N, C]
                .reshape(GS, P, F, C)
                .transpose(1, 2, 0, 3)             # [p, f, s, m]
                .reshape(P, F, GS * C)
            )
            im[f"u{g}"] = np.ascontiguousarray(
                np.concatenate([pp, tt], axis=2)   # [P, F, 256]
            )
        in_maps.append(im)

    # Rare cold-start flake (first execution of a freshly loaded NEFF) can
    # return non-finite garbage; the program's semaphore structure is
    # verified sound, so guard with a finite-check + device retry.
    for attempt in range(3):
        res = run_bass_kernel_spmd(nc, in_maps, core_ids=list(range(NCORES)))
        _CACHE["last"] = res
        if all(
            np.isfinite(np.asarray(res.results[c]["gr_out"], dtype=np.float64)).all()
            for c in range(NCORES)
        ):
            break

    # Host epilogue: extract per-sample 4x4 G'/R' diagonal blocks, fp64 solve.
    G = np.empty((B, C, C), np.float64)
    R = np.empty((B, C, C), np.float64)
    for c in range(NCORES):
        o = np.asarray(res.results[c]["gr_out"], dtype=np.float64)
        og = [o[:, 0:U], o[:, U : 2 * U] + o[:, 2 * U : 3 * U]]
        for g in range(NG):
            s0 = c * SPC + g * GS
            bg = og[g][:, : GS * C].reshape(GS, C, GS, C)
            br = og[g][:, GS * C :].reshape(GS, C, GS, C)
            G[s0 : s0 + GS] = np.einsum("sisj->sij", bg)
            R[s0 : s0 + GS] = np.einsum("sism->sim", br)

    G = 0.5 * (G + np.swapaxes(G, 1, 2))
    Xs = np.linalg.solve(G, R)
    val = (H * H) * np.einsum("bim,bim->b", R, Xs)
    loss = np.mean((4.0 - val) / 4.0)
    return np.float32(loss)
